# revision 12
# baseline (speedup 1.0000x reference)
"""Trainium2 Bass kernel for the pairwise concordance-index loss.

reference:
    loss = sum_{i<j, f_i=f_j=1} relu((p_i-p_j)(t_i-t_j)) / 100 / n_pairs

Math:
  Work only on the COMPACTED set (rows with f=1, m of them).
  M[i,j] = (p_i-p_j)(t_i-t_j) = A^T B, rank 4:
      A = [u, 1, p, t], B = [1, u, -t, -p], u = p*t.
  sum relu(M) = 0.5*(sum M + sum |M|); sum M has an O(m) closed form done
  on the host in fp64 over the same bf16 values the device uses; sum |M|
  is the O(m^2) part done on device.

Device decomposition (8 cores, identical program, data-sharded):
  Nb = ceil(m/128) row-blocks of 128; core k owns nb = ceil(Nb/8)
  consecutive block slots (slots past Nb hold zeros).  Each block
  processes its cyclic column window (offsets e=0..E wrapping mod
  Nb*128); the e=0 slab (and for even Nb the antipodal slab) is
  pre-scaled 0.5 on the host so all device sums have uniform weight.
  Per core the col stream (nb*C cols) is split into 4 equal per-quad
  streams; the 4 quads run as concurrent K=4 bf16 matmuls packed into
  disjoint 32-row PE groups via tile_position.  Each PSUM "sweep" fills
  4 banks (one per quad) = two [128,2,512] tiles (4 persistent tiles,
  ping-pong), consumed by abs-row-sum jobs alternating between the DVE
  (tensor_reduce apply_absolute_value) and the ScalarE (activation Abs +
  accum_out).  At the end a K=128 fp32 ones-matmul folds acc [128,NJ]
  -> [1,NJ], copied and DMA'd out as a single line.
  Input arrives as 4 per-quad [4,W] DMAs issued in parallel from 4
  different engine queues (2 column chunks each so the first sweep's
  data lands early).
"""

import numpy as np

B = 8192
P = 128
NCORE = 8

_cache = {}


def _plan(m):
    """Geometry from compacted count m."""
    Nb = -(-m // P)               # real cyclic block count
    nb = -(-Nb // NCORE)          # block slots per core
    if Nb % 2 == 0:
        C = (Nb // 2 + 1) * P     # offsets e=0..Nb/2; e=0 and e=Nb/2 at 0.5
    else:
        C = ((Nb + 1) // 2) * P   # offsets e=0..(Nb-1)/2; e=0 at 0.5
    Q = nb * C // 4               # cols per quad stream
    nsweep = -(-Q // 512)         # banks per quad
    WA = 3 * P                    # stationary slab area (3 block slots)
    W = WA + Q                    # line length
    return Nb, nb, C, Q, nsweep, WA, W


def _quad_slots(q, Q, C):
    """Distinct local block slots touched by quad q's stream, in order."""
    lo, hi = Q * q, Q * (q + 1)
    return list(range(lo // C, (hi - 1) // C + 1))


def _segments(q, Q, C, nsweep):
    """Per sweep: list of (bank_col0, ncols, slot, streampos) matmul segs."""
    slots = _quad_slots(q, Q, C)
    out = []
    for s in range(nsweep):
        b0, b1 = 512 * s, min(512 * (s + 1), Q)
        segs = []
        c = b0
        while c < b1:
            gpos = Q * q + c
            blk = gpos // C
            nxt = min(b1, (blk + 1) * C - Q * q)
            segs.append((c - b0, nxt - c, slots.index(blk), c))
            c = nxt
        out.append(segs)
    return out


def _build(m):
    """Build + compile the Bass module for compacted size m."""
    import concourse.bacc as bacc
    import concourse.tile as tile
    import concourse.mybir as mybir

    f32 = mybir.dt.float32
    bf16 = mybir.dt.bfloat16
    Nb, nb, C, Q, nsweep, WA, W = _plan(m)
    NJ = 2 * nsweep
    
    nc = bacc.Bacc("TRN2", target_bir_lowering=False, debug=False,
                   num_devices=NCORE)
    in_dram = nc.dram_tensor("inp", [4, 4, W], bf16, kind="ExternalInput")
    out_dram = nc.dram_tensor("acc", [1, NJ], f32, kind="ExternalOutput")

    segs_q = [_segments(q, Q, C, nsweep) for q in range(4)]

    with tile.TileContext(nc) as tc:
        with (
            tc.tile_pool(name="inp", bufs=1) as inp_pool,
            tc.tile_pool(name="ps", bufs=1, space="PSUM") as ps,
        ):
            sb = inp_pool.tile([P, W], bf16)
            dma_eng = [nc.gpsimd, nc.sync]
            cuts = (0, WA + 512, min(WA + 1536, W), W)
            for ci in range(3):
                c0, c1 = cuts[ci], cuts[ci + 1]
                if c0 >= c1:
                    continue
                for q in range(4):
                    dma_eng[q % 2].dma_start(
                        sb[32 * q:32 * q + 4, c0:c1], in_dram.ap()[q, :, c0:c1])

            acc = inp_pool.tile([P, NJ], f32)
            fold_sb = inp_pool.tile([1, NJ], f32)

            tiles = [ps.tile([P, 2, 512], f32, name=f"rot{i}")
                     for i in range(4)]
            # fold output reuses the first rotation tile's PSUM (all 8
            # banks are taken by the rotation tiles)
            fold_ps = tiles[0][0:1, 0:1, 0:NJ]

            job = 0

            def reduce_tile(red, use_dve):
                nonlocal job
                if use_dve:
                    nc.vector.tensor_reduce(
                        acc[:, job:job + 1], red,
                        axis=mybir.AxisListType.XY, op=mybir.AluOpType.add,
                        apply_absolute_value=True,
                    )
                else:
                    nc.scalar.activation(
                        red, red, mybir.ActivationFunctionType.Abs,
                        accum_out=acc[:, job:job + 1],
                    )
                job += 1

            for s in range(nsweep):
                ncols = min(512, Q - 512 * s)
                ta = tiles[2 * (s % 2)]
                tb = tiles[2 * (s % 2) + 1]
                for q in range(4):
                    t = (ta, tb)[q // 2]
                    for (c0, n, slot, spos) in segs_q[q][s]:
                        nc.tensor.matmul(
                            t[:, q % 2, c0:c0 + n],
                            sb[32 * q:32 * q + 4, P * slot:P * slot + P],
                            sb[32 * q:32 * q + 4, WA + spos:WA + spos + n],
                            start=True, stop=True,
                            tile_position=(32 * q, 0),
                        )
                for t in (ta, tb):
                    red = t[:, :, :] if ncols == 512 else t[:, :, 0:ncols]
                    # strict alternation so both engines start on sweep 0;
                    # the two final partial jobs land one on each engine.
                    reduce_tile(red, use_dve=(job % 2 == 0))

            assert job == NJ, job
            ones = nc.const_aps.tensor(1.0, [P, 1], f32)
            nc.tensor.matmul(fold_ps, ones, acc[:, :],
                             start=True, stop=True)
            nc.scalar.copy(fold_sb[:, :], fold_ps[:, :])
            nc.gpsimd.dma_start(out_dram.ap()[:, :], fold_sb[:, :])

    nc.compile()
    return nc


def _get_nc(m=None):
    if m is None:
        m = _cache["last_m"]
    key = _plan(m)[:2]
    if ("nc", key) not in _cache:
        _cache[("nc", key)] = _build(m)
    _cache["last_m"] = m
    return _cache[("nc", key)]


def _make_in_maps(p, t, f, u):
    """Compact by flag, build per-core [4,4,W] bf16 input arrays.

    Returns (in_maps, A_pad, B_pad) with the padded bf16 factor arrays the
    host closed form must use.
    """
    import ml_dtypes

    idx = np.nonzero(f != 0.0)[0]
    m = len(idx)
    Nb, nb, C, Q, nsweep, WA, W = _plan(m)
    m_cyc = Nb * P                # cyclic window modulus
    m_pad = NCORE * nb * P        # block-slot span (>= m_cyc)

    one = np.ones(m, np.float32)
    A = np.zeros((4, m_pad), dtype=ml_dtypes.bfloat16)
    Bm = np.zeros((4, m_pad), dtype=ml_dtypes.bfloat16)
    A[:, :m] = np.stack([u[idx], one, p[idx], t[idx]]).astype(ml_dtypes.bfloat16)
    Bm[:, :m] = np.stack([one, u[idx], -t[idx], -p[idx]]).astype(ml_dtypes.bfloat16)

    half = np.asarray(0.5, dtype=ml_dtypes.bfloat16)
    in_maps = []
    for k in range(NCORE):
        arr = np.zeros((4, 4, W), dtype=ml_dtypes.bfloat16)
        stream = np.zeros((4, nb * C), dtype=ml_dtypes.bfloat16)
        for l in range(nb):
            a = nb * k + l
            if a >= Nb:
                continue          # dummy slot, stays zero
            cols = (P * a + np.arange(C)) % m_cyc
            blockw = Bm[:, cols].copy()
            blockw[:, :P] *= half
            if Nb % 2 == 0:
                blockw[:, C - P:] *= half
            stream[:, C * l:C * (l + 1)] = blockw
        for q in range(4):
            arr[q, :, WA:] = stream[:, Q * q:Q * (q + 1)]
            for si, blk in enumerate(_quad_slots(q, Q, C)):
                a = nb * k + blk
                arr[q, :, P * si:P * si + P] = A[:, P * a:P * a + P]
        in_maps.append({"inp": arr})
    _cache["last_m"] = m
    return in_maps, A, Bm


def kernel(pred, gt, gt_fracTime, gt_ifMOF):
    from concourse import bass_utils

    pred = np.asarray(pred)
    gt = np.asarray(gt)
    ift = int(np.asarray(gt_fracTime))
    imf = int(np.asarray(gt_ifMOF))

    p = pred.astype(np.float32)
    t = gt[:, ift].astype(np.float32)
    f = (gt[:, imf] == 1).astype(np.float32)
    u = (p * t).astype(np.float32)

    in_maps, A, Bm = _make_in_maps(p, t, f, u)
    nc = _get_nc()
    res = bass_utils.run_bass_kernel_spmd(nc, in_maps, core_ids=list(range(NCORE)))

    # T = sum_{i<j} |M| (uniform weight; fold already summed partitions)
    T = 0.0
    for r in res.results:
        T += r["acc"].astype(np.float64).sum()

    # host closed form in fp64 over the same bf16 values the device used:
    # sum_{i<j} M = (sum_{i,j} M - sum_diag M) / 2
    A64 = A.astype(np.float64)
    B64 = Bm.astype(np.float64)
    S_all = (A64.sum(axis=1) * B64.sum(axis=1)).sum()
    D_diag = (A64 * B64).sum()
    S_half = (S_all - D_diag) / 2.0

    m = float(len(np.nonzero(f != 0.0)[0]))
    n_pairs = (m * m - m) / 2.0

    loss = 0.5 * (S_half + T) / 100.0 / n_pairs
    return np.asarray(np.float32(loss))


# revision 13
# speedup vs baseline: 1.0051x; 1.0051x over previous
"""Trainium2 Bass kernel for the pairwise concordance-index loss.

reference:
    loss = sum_{i<j, f_i=f_j=1} relu((p_i-p_j)(t_i-t_j)) / 100 / n_pairs

Math:
  Work only on the COMPACTED set (rows with f=1, m of them).
  M[i,j] = (p_i-p_j)(t_i-t_j) = A^T B, rank 4:
      A = [u, 1, p, t], B = [1, u, -t, -p], u = p*t.
  sum relu(M) = 0.5*(sum M + sum |M|); sum M has an O(m) closed form done
  on the host in fp64 over the same bf16 values the device uses; sum |M|
  is the O(m^2) part done on device.

Device decomposition (8 cores, identical program, data-sharded):
  Nb = ceil(m/128) row-blocks of 128; core k owns nb = ceil(Nb/8)
  consecutive block slots (slots past Nb hold zeros).  Each block
  processes its cyclic column window (offsets e=0..E wrapping mod
  Nb*128); the e=0 slab (and for even Nb the antipodal slab) is
  pre-scaled 0.5 on the host so all device sums have uniform weight.
  Per core the col stream (nb*C cols) is split into 4 equal per-quad
  streams; the 4 quads run as concurrent K=4 bf16 matmuls packed into
  disjoint 32-row PE groups via tile_position.  Each PSUM "sweep" fills
  4 banks (one per quad) = two [128,2,512] tiles (4 persistent tiles,
  ping-pong), consumed by abs-row-sum jobs alternating between the DVE
  (tensor_reduce apply_absolute_value) and the ScalarE (activation Abs +
  accum_out).  At the end a K=128 fp32 ones-matmul folds acc [128,NJ]
  -> [1,NJ], copied and DMA'd out as a single line.
  Input arrives as 4 per-quad [4,W] DMAs issued in parallel from 4
  different engine queues (2 column chunks each so the first sweep's
  data lands early).
"""

import numpy as np

B = 8192
P = 128
NCORE = 8

_cache = {}


def _plan(m):
    """Geometry from compacted count m."""
    Nb = -(-m // P)               # real cyclic block count
    nb = -(-Nb // NCORE)          # block slots per core
    if Nb % 2 == 0:
        C = (Nb // 2 + 1) * P     # offsets e=0..Nb/2; e=0 and e=Nb/2 at 0.5
    else:
        C = ((Nb + 1) // 2) * P   # offsets e=0..(Nb-1)/2; e=0 at 0.5
    Q = nb * C // 4               # cols per quad stream
    nsweep = -(-Q // 512)         # banks per quad
    WA = 3 * P                    # stationary slab area (3 block slots)
    W = WA + Q                    # line length
    return Nb, nb, C, Q, nsweep, WA, W


def _quad_slots(q, Q, C):
    """Distinct local block slots touched by quad q's stream, in order."""
    lo, hi = Q * q, Q * (q + 1)
    return list(range(lo // C, (hi - 1) // C + 1))


def _segments(q, Q, C, nsweep):
    """Per sweep: list of (bank_col0, ncols, slot, streampos) matmul segs."""
    slots = _quad_slots(q, Q, C)
    out = []
    for s in range(nsweep):
        b0, b1 = 512 * s, min(512 * (s + 1), Q)
        segs = []
        c = b0
        while c < b1:
            gpos = Q * q + c
            blk = gpos // C
            nxt = min(b1, (blk + 1) * C - Q * q)
            segs.append((c - b0, nxt - c, slots.index(blk), c))
            c = nxt
        out.append(segs)
    return out


def _build(m):
    """Build + compile the Bass module for compacted size m."""
    import concourse.bacc as bacc
    import concourse.tile as tile
    import concourse.mybir as mybir

    f32 = mybir.dt.float32
    bf16 = mybir.dt.bfloat16
    Nb, nb, C, Q, nsweep, WA, W = _plan(m)
    NJ = 2 * nsweep
    
    nc = bacc.Bacc("TRN2", target_bir_lowering=False, debug=False,
                   num_devices=NCORE)
    in_dram = nc.dram_tensor("inp", [4, 4, W], bf16, kind="ExternalInput")
    out_dram = nc.dram_tensor("acc", [1, NJ], f32, kind="ExternalOutput")

    segs_q = [_segments(q, Q, C, nsweep) for q in range(4)]

    with tile.TileContext(nc) as tc:
        with (
            tc.tile_pool(name="inp", bufs=1) as inp_pool,
            tc.tile_pool(name="ps", bufs=1, space="PSUM") as ps,
        ):
            sb = inp_pool.tile([P, W], bf16)
            dma_eng = [nc.gpsimd, nc.sync]
            cuts = (0, WA + 512, min(WA + 1536, W), W)
            for ci in range(3):
                c0, c1 = cuts[ci], cuts[ci + 1]
                if c0 >= c1:
                    continue
                for q in range(4):
                    dma_eng[q % 2].dma_start(
                        sb[32 * q:32 * q + 4, c0:c1], in_dram.ap()[q, :, c0:c1])

            acc = inp_pool.tile([P, NJ], f32)
            fold_sb = inp_pool.tile([1, NJ], f32)

            tiles = [ps.tile([P, 2, 512], f32, name=f"rot{i}")
                     for i in range(4)]
            # fold output reuses the first rotation tile's PSUM (all 8
            # banks are taken by the rotation tiles)
            fold_ps = tiles[0][0:1, 0:1, 0:NJ]

            job = 0

            def reduce_tile(red, use_dve):
                nonlocal job
                if use_dve:
                    nc.vector.tensor_reduce(
                        acc[:, job:job + 1], red,
                        axis=mybir.AxisListType.XY, op=mybir.AluOpType.add,
                        apply_absolute_value=True,
                    )
                else:
                    nc.scalar.activation(
                        red, red, mybir.ActivationFunctionType.Abs,
                        accum_out=acc[:, job:job + 1],
                    )
                job += 1

            for s in range(nsweep):
                ncols = min(512, Q - 512 * s)
                ta = tiles[2 * (s % 2)]
                tb = tiles[2 * (s % 2) + 1]
                for q in range(4):
                    t = (ta, tb)[q // 2]
                    for (c0, n, slot, spos) in segs_q[q][s]:
                        nc.tensor.matmul(
                            t[:, q % 2, c0:c0 + n],
                            sb[32 * q:32 * q + 4, P * slot:P * slot + P],
                            sb[32 * q:32 * q + 4, WA + spos:WA + spos + n],
                            start=True, stop=True,
                            tile_position=(32 * q, 0),
                        )
                for t in (ta, tb):
                    red = t[:, :, :] if ncols == 512 else t[:, :, 0:ncols]
                    # strict alternation so both engines start on sweep 0,
                    # except one late job shifts to the DVE (the ScalarE
                    # runs ~1.2x slower per job and would straggle).
                    reduce_tile(red, use_dve=(job % 2 == 0 or job == NJ - 3))

            assert job == NJ, job
            ones = nc.const_aps.tensor(1.0, [P, 1], f32)
            nc.tensor.matmul(fold_ps, ones, acc[:, :],
                             start=True, stop=True)
            nc.scalar.copy(fold_sb[:, :], fold_ps[:, :])
            nc.gpsimd.dma_start(out_dram.ap()[:, :], fold_sb[:, :])

    nc.compile()
    return nc


def _get_nc(m=None):
    if m is None:
        m = _cache["last_m"]
    key = _plan(m)[:2]
    if ("nc", key) not in _cache:
        _cache[("nc", key)] = _build(m)
    _cache["last_m"] = m
    return _cache[("nc", key)]


def _make_in_maps(p, t, f, u):
    """Compact by flag, build per-core [4,4,W] bf16 input arrays.

    Returns (in_maps, A_pad, B_pad) with the padded bf16 factor arrays the
    host closed form must use.
    """
    import ml_dtypes

    idx = np.nonzero(f != 0.0)[0]
    m = len(idx)
    Nb, nb, C, Q, nsweep, WA, W = _plan(m)
    m_cyc = Nb * P                # cyclic window modulus
    m_pad = NCORE * nb * P        # block-slot span (>= m_cyc)

    one = np.ones(m, np.float32)
    A = np.zeros((4, m_pad), dtype=ml_dtypes.bfloat16)
    Bm = np.zeros((4, m_pad), dtype=ml_dtypes.bfloat16)
    A[:, :m] = np.stack([u[idx], one, p[idx], t[idx]]).astype(ml_dtypes.bfloat16)
    Bm[:, :m] = np.stack([one, u[idx], -t[idx], -p[idx]]).astype(ml_dtypes.bfloat16)

    half = np.asarray(0.5, dtype=ml_dtypes.bfloat16)
    in_maps = []
    for k in range(NCORE):
        arr = np.zeros((4, 4, W), dtype=ml_dtypes.bfloat16)
        stream = np.zeros((4, nb * C), dtype=ml_dtypes.bfloat16)
        for l in range(nb):
            a = nb * k + l
            if a >= Nb:
                continue          # dummy slot, stays zero
            cols = (P * a + np.arange(C)) % m_cyc
            blockw = Bm[:, cols].copy()
            blockw[:, :P] *= half
            if Nb % 2 == 0:
                blockw[:, C - P:] *= half
            stream[:, C * l:C * (l + 1)] = blockw
        for q in range(4):
            arr[q, :, WA:] = stream[:, Q * q:Q * (q + 1)]
            for si, blk in enumerate(_quad_slots(q, Q, C)):
                a = nb * k + blk
                arr[q, :, P * si:P * si + P] = A[:, P * a:P * a + P]
        in_maps.append({"inp": arr})
    _cache["last_m"] = m
    return in_maps, A, Bm


def kernel(pred, gt, gt_fracTime, gt_ifMOF):
    from concourse import bass_utils

    pred = np.asarray(pred)
    gt = np.asarray(gt)
    ift = int(np.asarray(gt_fracTime))
    imf = int(np.asarray(gt_ifMOF))

    p = pred.astype(np.float32)
    t = gt[:, ift].astype(np.float32)
    f = (gt[:, imf] == 1).astype(np.float32)
    u = (p * t).astype(np.float32)

    in_maps, A, Bm = _make_in_maps(p, t, f, u)
    nc = _get_nc()
    res = bass_utils.run_bass_kernel_spmd(nc, in_maps, core_ids=list(range(NCORE)))

    # T = sum_{i<j} |M| (uniform weight; fold already summed partitions)
    T = 0.0
    for r in res.results:
        T += r["acc"].astype(np.float64).sum()

    # host closed form in fp64 over the same bf16 values the device used:
    # sum_{i<j} M = (sum_{i,j} M - sum_diag M) / 2
    A64 = A.astype(np.float64)
    B64 = Bm.astype(np.float64)
    S_all = (A64.sum(axis=1) * B64.sum(axis=1)).sum()
    D_diag = (A64 * B64).sum()
    S_half = (S_all - D_diag) / 2.0

    m = float(len(np.nonzero(f != 0.0)[0]))
    n_pairs = (m * m - m) / 2.0

    loss = 0.5 * (S_half + T) / 100.0 / n_pairs
    return np.asarray(np.float32(loss))
